# revision 16
# baseline (speedup 1.0000x reference)
"""Trainium2 Bass kernel for nn_CumulativeFFT.

out[b,t,d,k,c] = (1/sqrt(2048)) * cumsum_t( x[b,t,d] * tw[t,k,c] )

Sharding: 8 cores = batch(4) x time-half(2). Each core computes its
(1024, 256, 32, 2) output shard; the cross-half running offset is computed
on-device from an x_prev input (zeros for first-half cores, so the program
stays SPMD-uniform).

Per-core algorithm (T on 126-row blocks, m-major contribution layout):
  - Contributions C[s, m*256+d] = x[s,d] * tw[s,m] built by DVE
    tensor_scalar ops (per-partition scalar = twiddle column; 4x mode,
    ~127ns/op). C is split into two half-width tiles (m<32 / m>=32),
    2-deep each, so WAR hazards release at half-block granularity and
    builds for block j+1 can start while block j's matmuls still run.
  - Causal cumsum via one PE matmul per 512-col chunk with a constant
    lhsT: rows s<126 = upper-tri ones, rows 126/127 = all-ones "carry
    rows". C rows 126/127 hold the running carry split hi/lo in bf16 so
    the fp32 carry survives the bf16 rhs (16-row remainder block keeps
    its carry at rows 16/17 with a dedicated 18-row lhsT slice).
  - Block carries from tiny blocksum matmuls tw_blk^T @ x_blk, exclusive-
    scanned in fp32; the scan add runs on Pool, the hi/lo split on DVE,
    all emitted one block ahead of use. The cross-half offset (x_prev)
    chain runs on ACT/Pool so it never blocks DVE's build stream.
  - PSUM -> bf16 convert (x bf16(1/sqrt(2048))) split ACT(10)/Pool(6) of
    the 16 psum tiles per block; the m-major -> d-major reorder is free
    via the strided convert APs. DVE does only builds + hi/lo, so no
    engine's per-block work exceeds the ~11.5us/block DMA store time.
  - A ~6us warmup burst of dummy matmuls ramps the PE p-state while the
    input DMAs run, so the first real matmuls run at full clock.
  - Carry-row DMAs are emitted one block ahead of the stores in SP-queue
    order, so they land before block j's matmuls need them instead of
    queueing behind a 5.7us store transfer.

The kernel is DMA-bound: 32MB of output stores per core = ~93us at the
360GB/s per-core DMA roofline.
"""

import math
import sys

import numpy as np

sys.path.insert(0, "/opt/trn_rl_repo")

import ml_dtypes

BF16 = ml_dtypes.bfloat16

B, T, D, K = 4, 2048, 256, 32
M2 = 2 * K            # 64 (k,c) pairs
MH = M2 // 2          # 32 m's per C half-tile
NCORES = 8
TH = T // 2           # 1024 time steps per core
TB = 126              # time-block rows (partitions 0..125; 126/127 = carry)
NFULL = TH // TB      # 8
REM = TH - NFULL * TB # 16
NBLK = NFULL + 1      # 9
NPREV = TH // 128     # 8 (128-row blocks of the other half, for the offset)
WID = M2 * D          # 16384 = m-major row width (col = m*D + d)
WH = MH * D           # 8192 = half-tile width
PS_FREE = 512         # psum tile free width: 2 m's = 1 bank (ring depth 8)
NORM = float(np.float32(BF16(1.0 / math.sqrt(T))))
NWARM = 24           # PE warmup matmuls (free-128, ramp p-state during loads)

# Convert-engine split per block (32 psum tiles): A=ACT (17), P=Pool (12),
# D=DVE (3, woven into the build stream with 3 tiles of slack).
_P_AT = {1, 3, 6, 9, 11, 14, 17, 19, 22, 25, 27, 30}
_D_AT = {4, 12, 20}
CONV_ENG = "".join(
    "P" if n in _P_AT else ("D" if n in _D_AT else "A") for n in range(32)
)

_prog = None


def _twiddles_np():
    n = np.arange(T, dtype=np.float32)
    k = np.arange(K, dtype=np.float32)
    ang = np.float32(-2.0 * math.pi / T) * np.outer(n, k)   # (T, K) f32
    tw = np.stack([np.cos(ang), np.sin(ang)], axis=-1)       # (T, K, 2)
    return tw.reshape(T, M2).astype(BF16)                    # m = k*2 + c


def _build_program():
    import concourse.bass as bass
    import concourse.tile as tile
    from concourse import bacc, mybir

    ts = bass.ts
    bf = mybir.dt.bfloat16
    f32 = mybir.dt.float32

    nc = bacc.Bacc(
        "TRN2", target_bir_lowering=False, debug=False, num_devices=NCORES
    )
    xo_h = nc.dram_tensor("x_own", [TH, D], bf, kind="ExternalInput")
    xp_h = nc.dram_tensor("x_prev", [TH, D], bf, kind="ExternalInput")
    two_h = nc.dram_tensor("tw_own", [128, NBLK * M2], bf, kind="ExternalInput")
    two32_h = nc.dram_tensor(
        "tw_own32", [128, NBLK * M2], f32, kind="ExternalInput"
    )
    twp_h = nc.dram_tensor("tw_prev", [128, NPREV * M2], bf, kind="ExternalInput")
    ut_h = nc.dram_tensor("ut", [128, TB + REM], bf, kind="ExternalInput")
    out_h = nc.dram_tensor("out", [TH, WID], bf, kind="ExternalOutput")

    with tile.TileContext(nc) as tc:
        with (
            tc.tile_pool(name="const", bufs=1) as cpool,
            tc.tile_pool(name="carry", bufs=1) as carpool,
            tc.tile_pool(name="cbuf", bufs=2) as cbpool,
            tc.tile_pool(name="obuf", bufs=3) as obpool,
            tc.tile_pool(name="ps", bufs=8, space="PSUM") as pspool,
        ):
            # PE warmup: ramp the p-state while input DMAs stream in.
            warm = cpool.tile([128, 512], bf, tag="warm")
            nc.vector.memset(warm[:, :], 0)
            ps_w = pspool.tile([128, 512], f32, tag="ps")
            for _ in range(NWARM):
                nc.tensor.matmul(
                    ps_w[:, 0:128], warm[:, 0:128], warm[:, 0:128],
                    start=True, stop=True,
                )

            # Loads: x_prev/tw_prev first (they gate the offset->carry(0)
            # chain, ~2.5us of latency after landing); x_own block 0 and
            # tw32 next so DVE builds start ASAP; the rest stream behind.
            xp_wide = cpool.tile([128, NPREV * D], bf, tag="xpw")
            nc.sync.dma_start(
                xp_wide[:, :],
                xp_h[:, :].rearrange("(i p) d -> p i d", p=128),
            )
            twp_t = cpool.tile([128, NPREV * M2], bf, tag="twp")
            nc.sync.dma_start(twp_t[:], twp_h[:])
            two32_t = cpool.tile([128, NBLK * M2], f32, tag="two32")
            nc.sync.dma_start(two32_t[:], two32_h[:])
            xo_wide = cpool.tile([128, NFULL * D], bf, tag="xow")
            nc.sync.dma_start(
                xo_wide[0:TB, 0:D],
                xo_h[0:TB, :],
            )
            ut_t = cpool.tile([128, TB + REM], bf, tag="ut")
            nc.sync.dma_start(ut_t[:], ut_h[:])
            nc.sync.dma_start(
                xo_wide[0:TB, D:].rearrange("p (j d) -> p j d", d=D),
                xo_h[TB : NFULL * TB, :].rearrange("(j p) d -> p j d", p=TB),
            )
            xo_rem = cpool.tile([128, D], bf, tag="xor")
            nc.sync.dma_start(xo_rem[0:REM, :], xo_h[NFULL * TB : TH, :])
            two_t = cpool.tile([128, NBLK * M2], bf, tag="two")
            nc.sync.dma_start(two_t[:], two_h[:])
            xo_tiles = [
                xo_wide[:, ts(j, D)] for j in range(NFULL)
            ] + [xo_rem[:, :]]
            xp_tiles = [xp_wide[:, ts(i, D)] for i in range(NPREV)]

            # ---- carry state ----
            # Single bf16 carry row per block (C row 126): the fp32 scan is
            # rounded to bf16 once per block, well within the 2e-2 gate.
            # Matmul APs use 127 rows so C row 127 is never read.
            carries = carpool.tile([64, NBLK * D], f32, tag="car")
            hi_t = carpool.tile([64, NBLK * D], bf, tag="hi")

            # Cross-half offset -> carries(0), on ACT so DVE's queue stays a
            # pure build stream at startup.
            ps_off = pspool.tile([64, D], f32, tag="ps")
            for i in range(NPREV):
                nc.tensor.matmul(
                    ps_off[:, :],
                    twp_t[:, ts(i, M2)],
                    xp_tiles[i],
                    start=(i == 0),
                    stop=(i == NPREV - 1),
                )
            nc.scalar.copy(carries[:, 0:D], ps_off[:, :])
            nc.scalar.copy(hi_t[:, 0:D], carries[:, 0:D])

            # C half-tiles rotate 2-deep via the pool; block j+1's tiles are
            # allocated during iteration j so the carry-row DMAs into them
            # can be emitted (and land) a block ahead of their matmuls.
            def alloc_c(j):
                C_a = cbpool.tile([128, WH], bf, tag="CA", name=f"CA{j}")
                C_b = cbpool.tile([128, WH], bf, tag="CB", name=f"CB{j}")
                return C_a, C_b

            def emit_carry_dma(j, C_a, C_b):
                ch = 126 if j < NFULL else REM
                for C_h, mbase in ((C_a, 0), (C_b, MH)):
                    nc.sync.dma_start(
                        C_h[ch : ch + 1, :].rearrange("p (a b) -> p a b", a=MH),
                        hi_t[mbase : mbase + MH, ts(j, D)],
                    )

            nextC = alloc_c(0)
            emit_carry_dma(0, *nextC)

            # ---- phase C ----
            # Merged per-tile emission: build the two m's a psum tile needs,
            # then immediately emit its matmul and convert. Everything
            # pipelines at 2-m granularity: PE chases the DVE build stream,
            # converts chase PE, with the depth-8 psum ring as elasticity.
            NPS = WID // PS_FREE   # 32 psum tiles per block
            mm = PS_FREE // D      # 2 m's per psum tile
            for j in range(NBLK):
                rows = TB if j < NFULL else REM
                C_a, C_b = nextC
                o_t = obpool.tile([128, WID], bf, tag="O")

                def emit_tile(n, o_t=o_t, j=j, rows=rows, C_a=C_a, C_b=C_b):
                    C_h = C_a if n < NPS // 2 else C_b
                    base = 0 if n < NPS // 2 else WH
                    ps_t = pspool.tile([128, PS_FREE], f32, tag="ps", name="ps")
                    col = n * PS_FREE - base
                    if j < NFULL:
                        nc.tensor.matmul(
                            ps_t[:TB, :],
                            ut_t[0:127, 0:TB],
                            C_h[0:127, col : col + PS_FREE],
                            start=True,
                            stop=True,
                        )
                    else:
                        nc.tensor.matmul(
                            ps_t[:REM, :],
                            ut_t[0 : REM + 1, TB : TB + REM],
                            C_h[0 : REM + 1, col : col + PS_FREE],
                            start=True,
                            stop=True,
                        )
                    src_v = ps_t[:rows, :].rearrange("p (mm d) -> p d mm", mm=mm)
                    dst = o_t[:rows, :].rearrange("p (d mm) -> p d mm", mm=M2)[
                        :, :, n * mm : (n + 1) * mm
                    ]
                    eng = CONV_ENG[n]
                    if eng == "A":
                        nc.scalar.mul(dst, src_v, NORM)
                    elif eng == "D":
                        nc.vector.tensor_scalar_mul(dst, src_v, NORM)
                    else:
                        nc.gpsimd.tensor_scalar_mul(dst, src_v, NORM)

                for mi in range(M2):
                    C_h = C_a if mi < MH else C_b
                    mh = mi if mi < MH else mi - MH
                    nc.vector.tensor_scalar_mul(
                        C_h[0:rows, mh * D : (mh + 1) * D],
                        xo_tiles[j][0:rows],
                        two32_t[0:rows, j * M2 + mi : j * M2 + mi + 1],
                    )
                    # tile n needs builds 2n, 2n+1; emit it 3 tiles (6 m's)
                    # behind the build stream so its matmul is long done by
                    # the time any DVE-assigned convert reaches queue head.
                    if mi >= 7 and mi % 2 == 1:
                        emit_tile((mi - 7) // 2)
                    # next block's carry: blocksum (PE) -> scan add (Pool)
                    # -> bf16 round (DVE), mid-stream, a block ahead of use.
                    if mi == MH - 1 and j + 1 < NBLK:
                        bs = pspool.tile([64, D], f32, tag="ps")
                        nc.tensor.matmul(
                            bs[:, :],
                            two_t[0:TB, ts(j, M2)],
                            xo_tiles[j][0:TB],
                            start=True,
                            stop=True,
                        )
                        nc.gpsimd.tensor_add(
                            carries[:, ts(j + 1, D)],
                            carries[:, ts(j, D)],
                            bs[:, :],
                        )
                        nc.vector.tensor_copy(
                            hi_t[:, ts(j + 1, D)], carries[:, ts(j + 1, D)]
                        )
                for n in range(NPS - 3, NPS):
                    emit_tile(n)
                # carry rows for block j+1 land in SP-queue order BEFORE
                # block j's stores, a full block before they're read.
                if j + 1 < NBLK:
                    nextC = alloc_c(j + 1)
                    emit_carry_dma(j + 1, *nextC)
                # stores slice the full-width tile by d-range: both sides
                # contiguous (HBM col = d*64 + m)
                for qq in range(2):
                    nc.sync.dma_start(
                        out_h[
                            j * TB : j * TB + rows,
                            qq * (WID // 2) : (qq + 1) * (WID // 2),
                        ],
                        o_t[:rows, qq * (WID // 2) : (qq + 1) * (WID // 2)],
                    )
    nc.compile()
    return nc


def _host_inputs(x):
    tw = _twiddles_np()
    ut = np.zeros((128, TB + REM), dtype=BF16)
    ut[0:TB, 0:TB] = np.triu(np.ones((TB, TB), dtype=np.float32)).astype(BF16)
    ut[126:128, 0:TB] = 1
    ut[0:REM, TB : TB + REM] = np.triu(np.ones((REM, REM), dtype=np.float32)).astype(
        BF16
    )
    ut[REM : REM + 2, TB : TB + REM] = 1
    twp = np.zeros((128, NPREV * M2), dtype=BF16)
    for i in range(NPREV):
        twp[:, i * M2 : (i + 1) * M2] = tw[i * 128 : (i + 1) * 128, :]
    in_maps = []
    for c in range(NCORES):
        b, h = divmod(c, 2)
        base = h * TH
        xo = np.ascontiguousarray(x[b, base : base + TH, :])
        xp = (
            np.ascontiguousarray(x[b, 0:TH, :])
            if h
            else np.zeros((TH, D), dtype=BF16)
        )
        two = np.zeros((128, NBLK * M2), dtype=BF16)
        for j in range(NBLK):
            rows = TB if j < NFULL else REM
            two[0:rows, j * M2 : (j + 1) * M2] = tw[
                base + j * TB : base + j * TB + rows, :
            ]
        in_maps.append(
            {
                "x_own": xo,
                "x_prev": xp,
                "tw_own": two,
                "tw_own32": two.astype(np.float32),
                "tw_prev": twp,
                "ut": ut,
            }
        )
    return in_maps


def kernel(x):
    global _prog
    x = np.asarray(x)
    assert x.shape == (B, T, D), x.shape
    if x.dtype != BF16:
        x = x.astype(BF16)
    if _prog is None:
        _prog = _build_program()
    from concourse.bass_utils import run_bass_kernel_spmd

    in_maps = _host_inputs(x)
    res = run_bass_kernel_spmd(_prog, in_maps, list(range(NCORES)))
    out = np.empty((B, T, D, K, 2), dtype=BF16)
    for c in range(NCORES):
        b, h = divmod(c, 2)
        out[b, h * TH : (h + 1) * TH] = res.results[c]["out"].reshape(TH, D, K, 2)
    return out
